# revision 35
# baseline (speedup 1.0000x reference)
"""Trainium2 Bass kernel: adaptive focal loss (reduction='mean').

reference:
    logp  = log_softmax(logits, axis=1)          # [B, V]
    logpt = logp[r, target[r]]                   # [B]
    pt    = exp(logpt)
    gamma = 5 if pt < 0.2 else (3 if pt < 0.5 else 1)
    loss  = mean(-(1 - pt)**gamma * logpt)

Strategy (data-parallel over batch, 8 NeuronCores):
  Each core takes 256 rows of logits [2048, 50257] f32. Per 128-row tile
  it streams the 50257-wide row in 8192-col chunks (32KB/partition
  descriptors: under the 64KB SDMA limit, 2x fewer descriptors than 16KB
  so the single HW queue's dispatch keeps all 16 DMA engines fed),
  computing exp + free-dim accumulation in one ScalarE activation per
  chunk (no max subtraction: logits are O(1), sumexp ~1e5, well inside
  f32 range).

  The combined exp+ln activation table (act_func_sets[6]) is force-
  loaded once at the top via InstLoadActFuncSet, so the mid/tail Ln and
  Exp activations never pay a 1.3us table switch.

  The target logit per row is fetched with an indirect (gather) DMA on
  GpSimd's SWDGE queue. Nothing consumes tval until each tile's
  epilogue, which computes logpt = tval - ln(S) first and then
  pt = exp(logpt) - so the slow (~20us) gather is never on ScalarE's
  critical path. Tile 0's epilogue (ln, pt, gamma select, loss, store)
  runs mid-stream; only tile 1's epilogue trails the last chunk.

  Memory roofline per core: 256*50257*4B = 51.5 MB read; 16 DMA engines
  x ~26.9 GB/s = ~430 GB/s ceiling => ~120 us. ScalarE exp: 12.9M elem
  @ 153.6 G/s => ~84 us (hidden under the stream).
"""

import os
import numpy as np

B = 2048
V = 50257
N_CORES = 8
B_SHARD = B // N_CORES  # 256
P = 128
N_TILES = B_SHARD // P  # 2
CHUNK_W = 8192  # 32KB/partition per descriptor: measured 414 GB/s on the
# sync ring; 16-20KB descriptors measured ~17% slower (345 GB/s), and the
# scalar engine's HWDGE ring measured ~140 GB/s - so ALL streaming rides
# the sync ring with fat descriptors, with only two sub-8192 chunks per
# tile-1 tail. Stream order is cross-tile: tile 0's bulk, all of tile 1,
# then tile 0's tiny 1105-col chunk LAST - so the post-stream serial work
# is a 1.2us exp + epilogue instead of tile 1's 7us exp backlog
# (ScalarE exps run at 1148 cols/us vs DMA's 808.6 cols/us; sizes from a
# mini-sim over those measured rates).
CHUNK_SCHED_T = [
    [CHUNK_W] * 6 + [1105],
    [CHUNK_W] * 4 + [6144, 4600, 3200, 2048, 1024, 473],
]
assert all(sum(s) == V for s in CHUNK_SCHED_T)
N_CHUNKS_T = [len(s) for s in CHUNK_SCHED_T]  # [7, 10]
CH_OFF = [0, N_CHUNKS_T[0]]  # s_all column offset per tile
N_CHUNKS_TOT = sum(N_CHUNKS_T)
XBUFS = 6  # 6 x 32KB/partition = 192KB of the ~208KB budget

_PROGRAM = None
LAST_RESULTS = None  # BassKernelResults of the most recent run (for test harness)


def _install_axon_ntff_hook():
    """Make `antenv.axon_hooks` importable so trace=True works under axon.

    The agent image's antenv package lacks the axon_hooks shim that
    concourse's run_bass_kernel_spmd imports when tracing; inject an
    equivalent module backed by libaxon_pjrt.so's profile entry points.
    No-op if anything is missing; tracing then just degrades.
    """
    import sys
    import types

    if "antenv.axon_hooks" in sys.modules:
        return
    try:
        import antenv  # noqa: F401
    except Exception:
        return
    hook = None
    try:
        from trn_agent_boot.trn_boot import _ntff_profile_via_ctypes

        so_path = "/opt/axon/libaxon_pjrt.so"
        if os.path.exists(so_path):
            hook = _ntff_profile_via_ctypes(so_path)
    except Exception:
        hook = None
    try:
        mod = types.ModuleType("antenv.axon_hooks")
        _state = {"hook": hook}
        mod.set_axon_ntff_profile_hook = lambda h: _state.__setitem__("hook", h)
        mod.get_axon_ntff_profile_hook = lambda: _state["hook"]
        sys.modules["antenv.axon_hooks"] = mod
    except Exception:
        pass


def _build_program():
    from contextlib import ExitStack

    import concourse.bass as bass
    import concourse.mybir as mybir
    import concourse.tile as tile
    from concourse import bacc

    from concourse import bass_isa

    RED = bass_isa.ReduceOp
    f32 = mybir.dt.float32
    nc = bacc.Bacc(
        "TRN2",
        target_bir_lowering=False,
        debug=False,
        num_devices=N_CORES,
    )
    logits = nc.dram_tensor("logits", [B_SHARD, V], f32, kind="ExternalInput")
    tidx = nc.dram_tensor("tidx", [P, N_TILES], mybir.dt.int32, kind="ExternalInput")
    # Per-core output: one partial loss sum per row-tile (host sums all and
    # divides by B). A [P, NT] per-row loss store would be 128 4-byte
    # descriptors (~4.7us DMA); the per-tile scalar is one descriptor.
    out = nc.dram_tensor("out", [1, N_TILES], f32, kind="ExternalOutput")

    ACT = mybir.ActivationFunctionType
    ALU = mybir.AluOpType
    NT = N_TILES

    with tile.TileContext(nc) as tc, ExitStack() as ctx:
        xp = ctx.enter_context(tc.tile_pool(name="xp", bufs=XBUFS))
        sp = ctx.enter_context(tc.tile_pool(name="sp", bufs=1))

        # Force the combined exp+ln table (act_func_sets[6]) before any
        # activation: every Exp/Ln below is then served by the resident
        # table and the implicit-table-load pass inserts nothing.
        ld = mybir.InstLoadActFuncSet(
            name=nc.get_next_instruction_name(), ins=[], outs=[]
        )
        ld.act_func_set_id = 6
        nc.scalar.add_instruction(ld)

        # Gather logits[r, target[r]]: index load on the scalar HWDGE
        # queue (ScalarE is idle until the first chunk lands, and this
        # keeps the sync queue's ring pure chunk traffic), gathers on
        # GpSimd's SWDGE. Slow (~20us) but nothing consumes tval until
        # the per-tile epilogue.
        idxt = sp.tile([P, NT], mybir.dt.int32, tag="idx")
        nc.scalar.dma_start(idxt[:], tidx[:])
        tval = sp.tile([P, NT], f32, tag="tval")
        for t in range(NT):
            nc.gpsimd.indirect_dma_start(
                out=tval[:, t : t + 1],
                out_offset=None,
                in_=bass.AP(logits, 0, [[1, B_SHARD * V], [1, 1]]),
                in_offset=bass.IndirectOffsetOnAxis(ap=idxt[:, t : t + 1], axis=0),
            )

        s_all = sp.tile([P, N_CHUNKS_TOT], f32, tag="s_all")
        sscr = sp.tile([P, max(N_CHUNKS_T)], f32, tag="sscr")
        S = sp.tile([P, NT], f32, tag="S")
        lse = sp.tile([P, NT], f32, tag="lse")
        logpt = sp.tile([P, NT], f32, tag="logpt")
        pt = sp.tile([P, NT], f32, tag="pt")
        u = sp.tile([P, NT], f32, tag="u")
        u2 = sp.tile([P, NT], f32, tag="u2")
        u3 = sp.tile([P, NT], f32, tag="u3")
        u5 = sp.tile([P, NT], f32, tag="u5")
        m1 = sp.tile([P, NT], mybir.dt.uint8, tag="m1")
        m2 = sp.tile([P, NT], mybir.dt.uint8, tag="m2")
        powv = sp.tile([P, NT], f32, tag="powv")
        loss = sp.tile([P, NT], f32, tag="loss")
        lsum = sp.tile([P, NT], f32, tag="lsum")

        def epilogue(ts):
            """Per-tile tail: S -> lse -> pt -> gamma select -> loss ->
            store. Tile 0's copy runs mid-stream (store on idle GpSimd
            SWDGE so the sync ring stays pure chunk issues); tile 1's is
            the only work after the last chunk.

            The S/lse/pt head runs entirely on ScalarE, in-order with the
            last chunk's READ_ACCUMULATOR (no cross-engine hops): the
            chunk partials reduce via a Copy-activation's accumulator,
            and pt = exp(tval - lse) uses the activation's per-partition
            bias operand. logpt is computed on DVE in parallel with the
            pt exp. Exp/Ln hit the resident combined table (no reload);
            the gamma masks run on GpSimd alongside DVE's power chain."""
            t = ts.start
            nco = N_CHUNKS_T[t]
            nc.scalar.activation(
                sscr[:, :nco],
                s_all[:, CH_OFF[t] : CH_OFF[t] + nco],
                ACT.Copy,
                accum_out=S[:, ts],
            )
            nc.scalar.activation(lse[:, ts], S[:, ts], ACT.Ln)
            # pt = exp(-lse + tval); bias is a per-partition [P,1] AP
            nc.scalar.activation(
                pt[:, ts], lse[:, ts], ACT.Exp, bias=tval[:, ts], scale=-1.0
            )
            nc.vector.tensor_sub(logpt[:, ts], tval[:, ts], lse[:, ts])
            nc.vector.tensor_scalar(
                u[:, ts], pt[:, ts], -1.0, 1.0, op0=ALU.mult, op1=ALU.add
            )
            nc.vector.tensor_mul(u2[:, ts], u[:, ts], u[:, ts])
            nc.vector.tensor_mul(u3[:, ts], u2[:, ts], u[:, ts])
            nc.vector.tensor_mul(u5[:, ts], u2[:, ts], u3[:, ts])
            nc.gpsimd.tensor_scalar(m1[:, ts], pt[:, ts], 0.2, None, op0=ALU.is_lt)
            nc.gpsimd.tensor_scalar(m2[:, ts], pt[:, ts], 0.5, None, op0=ALU.is_lt)
            # gamma thresholds nest (pt<0.2 => pt<0.5), so two predicated
            # overwrites on top of the gamma=1 value select the power.
            nc.vector.tensor_copy(powv[:, ts], u[:, ts])
            nc.vector.copy_predicated(powv[:, ts], m2[:, ts], u3[:, ts])
            nc.vector.copy_predicated(powv[:, ts], m1[:, ts], u5[:, ts])
            # loss = -(1-pt)^gamma * logpt, then cross-partition sum on
            # GpSimd so the store is one 4-byte descriptor.
            nc.vector.scalar_tensor_tensor(
                loss[:, ts], in0=powv[:, ts], scalar=-1.0, in1=logpt[:, ts],
                op0=ALU.mult, op1=ALU.mult,
            )
            nc.gpsimd.partition_all_reduce(
                lsum[:, ts], loss[:, ts], channels=P, reduce_op=RED.add
            )
            if t == NT - 1:
                # Single 8B store of both tiles' partial sums, placed
                # last in the sync queue's program order so no chunk
                # issue ever waits behind it.
                nc.sync.dma_start(out[0:1, :], lsum[0:1, :])

        # Row-wise sum(exp(x)): chunked stream, exp+accumulate on ScalarE.
        for t in range(NT):
            r0 = t * P
            c0 = 0
            for c, w in enumerate(CHUNK_SCHED_T[t]):
                x = xp.tile([P, CHUNK_W], f32, tag="x")
                k = CH_OFF[t] + c
                nc.sync.dma_start(x[:, :w], logits[r0 : r0 + P, c0 : c0 + w])
                nc.scalar.activation(
                    x[:, :w], x[:, :w], ACT.Exp, accum_out=s_all[:, k : k + 1]
                )
                c0 += w
            if t == 0:
                epilogue(slice(0, 1))
        epilogue(slice(1, NT))

    nc.compile()
    return nc


def _get_program():
    global _PROGRAM
    if _PROGRAM is None:
        _PROGRAM = _build_program()
    return _PROGRAM


def kernel(**inputs) -> np.ndarray:
    global LAST_RESULTS

    logits = np.asarray(inputs["logits"], dtype=np.float32)
    target = np.asarray(inputs["target"]).astype(np.int64)
    assert logits.shape == (B, V), logits.shape
    assert target.shape == (B,), target.shape

    trace = bool(os.environ.get("KERNEL_TRACE")) or bool(os.environ.get("BASS_TRACE"))
    _install_axon_ntff_hook()

    in_maps = []
    for c in range(N_CORES):
        rows = slice(c * B_SHARD, (c + 1) * B_SHARD)
        shard = np.ascontiguousarray(logits[rows])
        tgt = target[rows]
        flat_idx = (
            (np.arange(B_SHARD, dtype=np.int64) * V + tgt)
            .astype(np.int32)
            .reshape(N_TILES, P)
            .T  # [P, N_TILES]: column t = rows of row-tile t
        )
        in_maps.append({"logits": shard, "tidx": np.ascontiguousarray(flat_idx)})

    from concourse.bass_utils import run_bass_kernel_spmd

    nc = _get_program()
    res = run_bass_kernel_spmd(
        nc, in_maps, core_ids=list(range(N_CORES)), trace=trace
    )
    LAST_RESULTS = res

    total = np.float64(0.0)
    for c in range(N_CORES):
        # each core returns its two row-tile partial loss sums
        total += np.asarray(res.results[c]["out"], dtype=np.float64).sum()
    return np.asarray(np.float32(total / B))


if __name__ == "__main__":
    rng = np.random.default_rng(0)
    logits = rng.standard_normal((B, V), dtype=np.float32)
    target = rng.integers(0, V, size=(B,)).astype(np.int64)
    out = kernel(logits=logits, target=target)
    print("kernel out:", out)


# revision 38
# speedup vs baseline: 1.0362x; 1.0362x over previous
"""Trainium2 Bass kernel: adaptive focal loss (reduction='mean').

reference:
    logp  = log_softmax(logits, axis=1)          # [B, V]
    logpt = logp[r, target[r]]                   # [B]
    pt    = exp(logpt)
    gamma = 5 if pt < 0.2 else (3 if pt < 0.5 else 1)
    loss  = mean(-(1 - pt)**gamma * logpt)

Strategy (data-parallel over batch, 8 NeuronCores):
  Each core takes 256 rows of logits [2048, 50257] f32. Per 128-row tile
  it streams the 50257-wide row in 8192-col chunks (32KB/partition
  descriptors: under the 64KB SDMA limit, 2x fewer descriptors than 16KB
  so the single HW queue's dispatch keeps all 16 DMA engines fed),
  computing exp + free-dim accumulation in one ScalarE activation per
  chunk (no max subtraction: logits are O(1), sumexp ~1e5, well inside
  f32 range).

  The combined exp+ln activation table (act_func_sets[6]) is force-
  loaded once at the top via InstLoadActFuncSet, so the mid/tail Ln and
  Exp activations never pay a 1.3us table switch.

  The target logit per row is fetched with an indirect (gather) DMA on
  GpSimd's SWDGE queue. Nothing consumes tval until each tile's
  epilogue, which computes logpt = tval - ln(S) first and then
  pt = exp(logpt) - so the slow (~20us) gather is never on ScalarE's
  critical path. Tile 0's epilogue (ln, pt, gamma select, loss, store)
  runs mid-stream; only tile 1's epilogue trails the last chunk.

  Memory roofline per core: 256*50257*4B = 51.5 MB read; 16 DMA engines
  x ~26.9 GB/s = ~430 GB/s ceiling => ~120 us. ScalarE exp: 12.9M elem
  @ 153.6 G/s => ~84 us (hidden under the stream).
"""

import os
import numpy as np

B = 2048
V = 50257
N_CORES = 8
B_SHARD = B // N_CORES  # 256
P = 128
N_TILES = B_SHARD // P  # 2
CHUNK_W = 8192  # 32KB/partition per descriptor: measured 414 GB/s on the
# sync ring; 16-20KB descriptors measured ~17% slower (345 GB/s), and the
# scalar engine's HWDGE ring measured ~140 GB/s - so ALL streaming rides
# the sync ring with fat descriptors, with only two sub-8192 chunks per
# tile-1 tail. Stream order is cross-tile: tile 0's bulk, all of tile 1,
# then tile 0's tiny 1105-col chunk LAST - so the post-stream serial work
# is a 1.2us exp + epilogue instead of tile 1's 7us exp backlog
# (ScalarE exps run at 1148 cols/us vs DMA's 808.6 cols/us; sizes from a
# mini-sim over those measured rates).
CHUNK_SCHED_T = [
    [CHUNK_W] * 6 + [1105],
    [CHUNK_W] * 4 + [6144, 4600, 3200, 2048, 1024, 473],
]
assert all(sum(s) == V for s in CHUNK_SCHED_T)
N_CHUNKS_T = [len(s) for s in CHUNK_SCHED_T]  # [7, 10]
CH_OFF = [0, N_CHUNKS_T[0]]  # s_all column offset per tile
N_CHUNKS_TOT = sum(N_CHUNKS_T)
XBUFS = 6  # 6 x 32KB/partition = 192KB of the ~208KB budget

_PROGRAM = None
LAST_RESULTS = None  # BassKernelResults of the most recent run (for test harness)


def _install_axon_ntff_hook():
    """Make `antenv.axon_hooks` importable so trace=True works under axon.

    The agent image's antenv package lacks the axon_hooks shim that
    concourse's run_bass_kernel_spmd imports when tracing; inject an
    equivalent module backed by libaxon_pjrt.so's profile entry points.
    No-op if anything is missing; tracing then just degrades.
    """
    import sys
    import types

    if "antenv.axon_hooks" in sys.modules:
        return
    try:
        import antenv  # noqa: F401
    except Exception:
        return
    hook = None
    try:
        from trn_agent_boot.trn_boot import _ntff_profile_via_ctypes

        so_path = "/opt/axon/libaxon_pjrt.so"
        if os.path.exists(so_path):
            hook = _ntff_profile_via_ctypes(so_path)
    except Exception:
        hook = None
    try:
        mod = types.ModuleType("antenv.axon_hooks")
        _state = {"hook": hook}
        mod.set_axon_ntff_profile_hook = lambda h: _state.__setitem__("hook", h)
        mod.get_axon_ntff_profile_hook = lambda: _state["hook"]
        sys.modules["antenv.axon_hooks"] = mod
    except Exception:
        pass


def _build_program():
    from contextlib import ExitStack

    import concourse.bass as bass
    import concourse.mybir as mybir
    import concourse.tile as tile
    from concourse import bacc

    from concourse import bass_isa

    RED = bass_isa.ReduceOp
    f32 = mybir.dt.float32
    nc = bacc.Bacc(
        "TRN2",
        target_bir_lowering=False,
        debug=False,
        num_devices=N_CORES,
    )
    logits = nc.dram_tensor("logits", [B_SHARD, V], f32, kind="ExternalInput")
    tidx = nc.dram_tensor("tidx", [P, N_TILES], mybir.dt.int32, kind="ExternalInput")
    # Per-core output: tile 0's per-row losses land in col 0 (its 128-
    # descriptor store hides mid-stream on the scalar ring - and a mid-
    # stream store is load-bearing: schedules without one consistently
    # degrade the chunk stream). Tile 1's partial sum is pre-reduced on
    # GpSimd so the end-of-kernel store is ONE 4-byte descriptor
    # (a [P,1] store there costs ~4.7us) - host reads out[0,1] only.
    out = nc.dram_tensor("out", [P, N_TILES], f32, kind="ExternalOutput")

    ACT = mybir.ActivationFunctionType
    ALU = mybir.AluOpType
    NT = N_TILES

    with tile.TileContext(nc) as tc, ExitStack() as ctx:
        xp = ctx.enter_context(tc.tile_pool(name="xp", bufs=XBUFS))
        sp = ctx.enter_context(tc.tile_pool(name="sp", bufs=1))

        # Force the combined exp+ln table (act_func_sets[6]) before any
        # activation: every Exp/Ln below is then served by the resident
        # table and the implicit-table-load pass inserts nothing.
        ld = mybir.InstLoadActFuncSet(
            name=nc.get_next_instruction_name(), ins=[], outs=[]
        )
        ld.act_func_set_id = 6
        nc.scalar.add_instruction(ld)

        # Gather logits[r, target[r]]: index load on the scalar HWDGE
        # queue (ScalarE is idle until the first chunk lands, and this
        # keeps the sync queue's ring pure chunk traffic), gathers on
        # GpSimd's SWDGE. Slow (~20us) but nothing consumes tval until
        # the per-tile epilogue.
        idxt = sp.tile([P, NT], mybir.dt.int32, tag="idx")
        nc.scalar.dma_start(idxt[:], tidx[:])
        tval = sp.tile([P, NT], f32, tag="tval")
        for t in range(NT):
            nc.gpsimd.indirect_dma_start(
                out=tval[:, t : t + 1],
                out_offset=None,
                in_=bass.AP(logits, 0, [[1, B_SHARD * V], [1, 1]]),
                in_offset=bass.IndirectOffsetOnAxis(ap=idxt[:, t : t + 1], axis=0),
            )

        s_all = sp.tile([P, N_CHUNKS_TOT], f32, tag="s_all")
        sscr = sp.tile([P, max(N_CHUNKS_T)], f32, tag="sscr")
        S = sp.tile([P, NT], f32, tag="S")
        lse = sp.tile([P, NT], f32, tag="lse")
        logpt = sp.tile([P, NT], f32, tag="logpt")
        pt = sp.tile([P, NT], f32, tag="pt")
        u = sp.tile([P, NT], f32, tag="u")
        u2 = sp.tile([P, NT], f32, tag="u2")
        u3 = sp.tile([P, NT], f32, tag="u3")
        u5 = sp.tile([P, NT], f32, tag="u5")
        m1 = sp.tile([P, NT], mybir.dt.uint8, tag="m1")
        m2 = sp.tile([P, NT], mybir.dt.uint8, tag="m2")
        powv = sp.tile([P, NT], f32, tag="powv")
        loss = sp.tile([P, NT], f32, tag="loss")
        lsum = sp.tile([P, NT], f32, tag="lsum")

        def epilogue(ts):
            """Per-tile tail: S -> lse -> pt -> gamma select -> loss ->
            store. Tile 0's copy runs mid-stream (store on idle GpSimd
            SWDGE so the sync ring stays pure chunk issues); tile 1's is
            the only work after the last chunk.

            The S/lse/pt head runs entirely on ScalarE, in-order with the
            last chunk's READ_ACCUMULATOR (no cross-engine hops): the
            chunk partials reduce via a Copy-activation's accumulator,
            and pt = exp(tval - lse) uses the activation's per-partition
            bias operand. logpt is computed on DVE in parallel with the
            pt exp. Exp/Ln hit the resident combined table (no reload);
            the gamma masks run on GpSimd alongside DVE's power chain."""
            t = ts.start
            nco = N_CHUNKS_T[t]
            nc.scalar.activation(
                sscr[:, :nco],
                s_all[:, CH_OFF[t] : CH_OFF[t] + nco],
                ACT.Copy,
                accum_out=S[:, ts],
            )
            nc.scalar.activation(lse[:, ts], S[:, ts], ACT.Ln)
            # pt = exp(-lse + tval); bias is a per-partition [P,1] AP
            nc.scalar.activation(
                pt[:, ts], lse[:, ts], ACT.Exp, bias=tval[:, ts], scale=-1.0
            )
            nc.vector.tensor_sub(logpt[:, ts], tval[:, ts], lse[:, ts])
            nc.vector.tensor_scalar(
                u[:, ts], pt[:, ts], -1.0, 1.0, op0=ALU.mult, op1=ALU.add
            )
            nc.vector.tensor_mul(u2[:, ts], u[:, ts], u[:, ts])
            nc.vector.tensor_mul(u3[:, ts], u2[:, ts], u[:, ts])
            nc.vector.tensor_mul(u5[:, ts], u2[:, ts], u3[:, ts])
            nc.gpsimd.tensor_scalar(m1[:, ts], pt[:, ts], 0.2, None, op0=ALU.is_lt)
            nc.gpsimd.tensor_scalar(m2[:, ts], pt[:, ts], 0.5, None, op0=ALU.is_lt)
            # gamma thresholds nest (pt<0.2 => pt<0.5), so two predicated
            # overwrites on top of the gamma=1 value select the power.
            nc.vector.tensor_copy(powv[:, ts], u[:, ts])
            nc.vector.copy_predicated(powv[:, ts], m2[:, ts], u3[:, ts])
            nc.vector.copy_predicated(powv[:, ts], m1[:, ts], u5[:, ts])
            # loss = -(1-pt)^gamma * logpt
            nc.vector.scalar_tensor_tensor(
                loss[:, ts], in0=powv[:, ts], scalar=-1.0, in1=logpt[:, ts],
                op0=ALU.mult, op1=ALU.mult,
            )
            if t == 0:
                nc.scalar.dma_start(out[:, ts], loss[:, ts])
            else:
                nc.gpsimd.partition_all_reduce(
                    lsum[:, ts], loss[:, ts], channels=P, reduce_op=RED.add
                )
                nc.sync.dma_start(out[0:1, ts], lsum[0:1, ts])

        # Row-wise sum(exp(x)): chunked stream, exp+accumulate on ScalarE.
        for t in range(NT):
            r0 = t * P
            c0 = 0
            for c, w in enumerate(CHUNK_SCHED_T[t]):
                x = xp.tile([P, CHUNK_W], f32, tag="x")
                k = CH_OFF[t] + c
                nc.sync.dma_start(x[:, :w], logits[r0 : r0 + P, c0 : c0 + w])
                nc.scalar.activation(
                    x[:, :w], x[:, :w], ACT.Exp, accum_out=s_all[:, k : k + 1]
                )
                c0 += w
            if t == 0:
                epilogue(slice(0, 1))
        epilogue(slice(1, NT))

    nc.compile()
    return nc


def _get_program():
    global _PROGRAM
    if _PROGRAM is None:
        _PROGRAM = _build_program()
    return _PROGRAM


def kernel(**inputs) -> np.ndarray:
    global LAST_RESULTS

    logits = np.asarray(inputs["logits"], dtype=np.float32)
    target = np.asarray(inputs["target"]).astype(np.int64)
    assert logits.shape == (B, V), logits.shape
    assert target.shape == (B,), target.shape

    trace = bool(os.environ.get("KERNEL_TRACE")) or bool(os.environ.get("BASS_TRACE"))
    _install_axon_ntff_hook()

    in_maps = []
    for c in range(N_CORES):
        rows = slice(c * B_SHARD, (c + 1) * B_SHARD)
        shard = np.ascontiguousarray(logits[rows])
        tgt = target[rows]
        flat_idx = (
            (np.arange(B_SHARD, dtype=np.int64) * V + tgt)
            .astype(np.int32)
            .reshape(N_TILES, P)
            .T  # [P, N_TILES]: column t = rows of row-tile t
        )
        in_maps.append({"logits": shard, "tidx": np.ascontiguousarray(flat_idx)})

    from concourse.bass_utils import run_bass_kernel_spmd

    nc = _get_program()
    res = run_bass_kernel_spmd(
        nc, in_maps, core_ids=list(range(N_CORES)), trace=trace
    )
    LAST_RESULTS = res

    total = np.float64(0.0)
    for c in range(N_CORES):
        o = np.asarray(res.results[c]["out"], dtype=np.float64)
        # col 0: tile 0's per-row losses; out[0,1]: tile 1's partial sum
        total += o[:, 0].sum() + o[0, 1]
    return np.asarray(np.float32(total / B))


if __name__ == "__main__":
    rng = np.random.default_rng(0)
    logits = rng.standard_normal((B, V), dtype=np.float32)
    target = rng.integers(0, V, size=(B,)).astype(np.int64)
    out = kernel(logits=logits, target=target)
    print("kernel out:", out)
